# revision 23
# baseline (speedup 1.0000x reference)
"""Trainium2 Bass kernel for a DiffusionInteractionBlock (GNN message passing).

Edge-parallel across 8 cores with replicated node data/weights:
  - Host: sort edges by receiver; split nodes into 8 equal contiguous ranges
    (1250 nodes/core); each core owns the edges whose receiver falls in its
    range. Within a core, nodes go in groups of 128; each group's edge list
    is padded to a uniform number of 128-edge tiles so all cores run one
    identical (SPMD) program. Scatter indicators (both orientations) are
    prebuilt on host and DMA'd.
  - Device per core:
      * table phase: VUP[n] = v_in[n]_i @ (inv*W_up1) in fp16, batched big
        DMAs (V16T fully SBUF-resident during the phase).
      * per 128-edge tile: transpose-gather s16^T[sender] (MLP operand
        layout), gather VUP[sender] (edge-major). MLP on TensorE; the
        receiver-scalar contribution comes from a per-group
        ZGT = s16_grp^T @ WFR matmul expanded per tile through the hosted
        transposed indicator (no per-edge receiver gather). s_up[sender]
        comes from sstT @ WUP0 per tile (no SUP table). Tensor product on
        Vector (fused broadcast ops) + Scalar (Silu); scatter-add via
        indicator matmul into a per-node-group PSUM accumulator.
      * group epilogue: output linears and write the group's output rows.
  - Host: concatenate the 8 per-core output slices.

fp16 for gathered operands / MLP operands / scatter; accumulation in fp32
PSUM. W_mlp3 is pre-scaled by 256 (undone in the final assembly copy) to
keep tensor-product weights out of fp16 denormal range.
"""

import math
import sys

sys.path.insert(0, "/opt/trn_rl_repo")

import numpy as np

N_NODES = 10000
N_EDGES = 160000
MUL = 128
N_RADIAL = 8
AVG_NEIGH = 16.0
INV_SQRT3 = float(1.0 / np.sqrt(3.0))
N_CORES = 8
W3_SCALE = 256.0

_COMPILED = {}


def _prep(node_feats, edge_attrs, edge_feats, lengths, edge_index,
          W_scalar, W_up0, W_up1, W_mlp1, b_mlp1, W_mlp2, b_mlp2, W_mlp3,
          W_out0, W_out1):
    """Host-side sharding / layout prep. Returns (meta, per-core in_maps)."""
    f16 = np.float16
    f32 = np.float32

    N, E, C = N_NODES, N_EDGES, N_CORES
    NPC = N // C                       # nodes per core
    G = math.ceil(NPC / 128)           # node groups per core
    NG_LAST = NPC - (G - 1) * 128      # nodes in last group

    sender = np.asarray(edge_index[0]).astype(np.int64)
    receiver = np.asarray(edge_index[1]).astype(np.int64)
    perm = np.argsort(receiver, kind="stable")
    s_sorted = sender[perm]
    r_sorted = receiver[perm]

    ea = np.asarray(edge_attrs, f32)[perm]
    ef = np.asarray(edge_feats, f32)[perm]
    ln = np.asarray(lengths, f32)[perm]

    # bin-pack nodes into C*G groups of <=128 slots to balance per-group
    # edge counts (minimizes TPG, the padded tiles-per-group)
    import heapq
    deg = np.bincount(r_sorted, minlength=N)
    h = [(0, k) for k in range(C * G)]
    heapq.heapify(h)
    gn = np.zeros(C * G, np.int64)
    slot_c = np.zeros(N, np.int64)
    slot_g = np.zeros(N, np.int64)
    slot_l = np.zeros(N, np.int64)
    for n in np.argsort(-deg):
        while True:
            e, k = heapq.heappop(h)
            if gn[k] < 128:
                break
        slot_c[n] = k // G
        slot_g[n] = k % G
        slot_l[n] = gn[k]
        gn[k] += 1
        heapq.heappush(h, (e + int(deg[n]), k))

    core_of = slot_c[r_sorted]
    grp = slot_g[r_sorted]
    lid = slot_l[r_sorted]             # local node id within group (0..127)
    # sort by (core, group, sender): ascending-sender gathers are
    # page-friendly in HBM (edge order within a group is free)
    resort = np.lexsort((s_sorted, core_of * G + grp))
    s_sorted = s_sorted[resort]
    core_of, grp, lid = core_of[resort], grp[resort], lid[resort]
    ea, ef, ln = ea[resort], ef[resort], ln[resort]

    counts = np.zeros((C, G), np.int64)
    np.add.at(counts, (core_of, grp), 1)
    TPG = int(math.ceil(counts.max() / 128.0))
    EPG = TPG * 128                    # padded edges per group
    T = G * TPG                        # tiles per core
    Ec = T * 128                       # padded edges per core

    cum = np.concatenate([[0], np.cumsum(counts.reshape(-1))])

    snd = np.zeros((C, Ec), np.int16)
    rl = np.zeros((C, Ec), np.int16)   # local node id (indicator column)
    vmask = np.zeros((C, Ec), bool)
    yat = np.zeros((C, Ec, 4), f32)
    efl = np.zeros((C, 16, Ec), f16)

    for c in range(C):
        for g in range(G):
            k = c * G + g
            src0, src1 = cum[k], cum[k + 1]
            L = src1 - src0
            d0 = g * EPG
            snd[c, d0:d0 + L] = s_sorted[src0:src1].astype(np.int16)
            rl[c, d0:d0 + L] = lid[src0:src1].astype(np.int16)
            assert np.all(core_of[src0:src1] == c) and np.all(grp[src0:src1] == g)
            vmask[c, d0:d0 + L] = True
            yat[c, d0:d0 + L, :] = ea[src0:src1]
            efl[c, 0:8, d0:d0 + L] = ef[src0:src1].astype(f16).T
            efl[c, 8, d0:d0 + L] = ln[src0:src1, 0].astype(f16)

    def wrap_idx(a):                   # [C, Ec] -> [C, 128, Ec//16] int16
        w = a.reshape(C, Ec // 16, 16).transpose(0, 2, 1)
        return np.tile(w, (1, 8, 1)).copy()

    sidx = wrap_idx(snd)
    yattT = yat.reshape(C, T, 128, 4).transpose(0, 2, 1, 3).reshape(C, 128, T * 4).copy()

    # scatter indicators, both orientations, hosted:
    # A[c, t, e, n] = (lid == n) & valid
    A = (rl.reshape(C, T, 128)[:, :, :, None] == np.arange(128)[None, None, None, :])
    A &= vmask.reshape(C, T, 128)[:, :, :, None]
    try:
        from ml_dtypes import float8_e4m3fn as f8
    except ImportError:
        f8 = f16
    ind = A.transpose(0, 2, 1, 3).reshape(C, 128, T * 128).astype(f8)
    indT = A.transpose(0, 3, 1, 2).reshape(C, 128, T * 128).astype(f8)

    nf = np.asarray(node_feats, f32)
    S16 = nf[:, :MUL].astype(f16)
    NPAD = (math.ceil(N / 128)) * 128
    CONC = np.zeros((NPAD, 4 * MUL), f16)
    CONC[:N, :MUL] = S16
    # v, i-major, transposed: row (i*128+q) = v_in[:, q, i] over nodes
    V16T = np.ascontiguousarray(
        nf[:, MUL:].reshape(N, MUL, 3).transpose(2, 1, 0).reshape(3 * MUL, N)
    ).astype(f16)
    S16T = np.ascontiguousarray(S16.T)
    # per-core receiver-group columns, slot-ordered (zero for empty slots)
    S16TG = np.zeros((C, MUL, G * 128), f16)
    slot_full = slot_g * 128 + slot_l
    S16TG[slot_c, :, slot_full] = S16T.T    # fancy-index: [N, MUL] rows
    # output row (in [C, G*128] layout) for each node
    out_row = slot_c * (G * 128) + slot_full

    inv = 1.0 / np.sqrt(MUL)
    inv2 = 1.0 / np.sqrt(2 * MUL)
    oscl = inv2 / AVG_NEIGH

    W_scalar = np.asarray(W_scalar, f32)
    W_mlp1 = np.asarray(W_mlp1, f32)
    W_mlp2 = np.asarray(W_mlp2, f32)
    W_mlp3 = np.asarray(W_mlp3, f32)
    W_up0 = np.asarray(W_up0, f32)
    W_up1 = np.asarray(W_up1, f32)
    W_out0 = np.asarray(W_out0, f32)
    W_out1 = np.asarray(W_out1, f32)

    WFS = (inv * W_scalar @ W_mlp1[:MUL]).astype(f16)
    WFR = (inv * W_scalar @ W_mlp1[MUL:2 * MUL]).astype(f16)
    W1CD = np.zeros((16, MUL), f16)
    W1CD[:N_RADIAL + 1] = W_mlp1[2 * MUL:].astype(f16)
    W2 = W_mlp2.astype(f16)
    W3F = (W_mlp3 * W3_SCALE).astype(f32)
    W3F[:, 3 * MUL:] *= INV_SQRT3
    W3F = W3F.astype(f16)
    WUP0 = (inv * W_up0).astype(f16)
    WUP1 = (inv * W_up1).astype(f16)
    WOSA = (W_out0[:MUL] * oscl).astype(f16)
    WOSB = (W_out0[MUL:] * oscl).astype(f16)
    WOVA = (W_out1[:MUL] * oscl).astype(f16)
    WOVB = (W_out1[MUL:] * oscl).astype(f16)
    assert np.all(np.asarray(b_mlp1) == 0.0) and np.all(np.asarray(b_mlp2) == 0.0), \
        "kernel assumes zero MLP biases (silu via AF.Silu)"

    weights = dict(WFS=WFS, WFR=WFR, W1CD=W1CD, W2=W2, W3F=W3F, WUP0=WUP0,
                   WUP1=WUP1, WOSA=WOSA, WOSB=WOSB, WOVA=WOVA, WOVB=WOVB)

    in_maps = []
    for c in range(C):
        m = dict(CONC=CONC, V16T=V16T, S16TG=S16TG[c],
                 SIDX=sidx[c], IND=ind[c], INDT=indT[c],
                 YATT=yattT[c], EFLT=efl[c], **weights)
        in_maps.append(m)

    meta = dict(N=N, NPC=NPC, G=G, TPG=TPG, T=T, Ec=Ec, NG_LAST=NG_LAST,
                out_row=out_row)
    return meta, in_maps


def _build(meta):
    """Build the (single, SPMD) bass program for one core's work."""
    import concourse.bass as bass
    import concourse.tile as tile
    from concourse import bacc, mybir

    f16 = mybir.dt.float16
    f32 = mybir.dt.float32
    i16 = mybir.dt.int16
    f8 = mybir.dt.float8e4
    AF = mybir.ActivationFunctionType
    OP = mybir.AluOpType

    N, NPC, G, TPG, T, Ec = meta["N"], meta["NPC"], meta["G"], meta["TPG"], meta["T"], meta["Ec"]
    NG_LAST = meta["NG_LAST"]
    EPG = TPG * 128
    NCH = math.ceil(N / 128)           # 79 node chunks for the VUP table
    NPAD = NCH * 128                   # padded table rows
    NLAST = N - (NCH - 1) * 128        # rows in last chunk (16)

    nc = bacc.Bacc("TRN2", target_bir_lowering=False, debug=False,
                   num_devices=N_CORES)

    def din(name, shape, dt):
        return nc.dram_tensor(name, shape, dt, kind="ExternalInput").ap()

    CONC = din("CONC", [NCH * 128, 4 * MUL], f16)
    V16T = din("V16T", [3 * MUL, N], f16)
    S16TG = din("S16TG", [MUL, G * 128], f16)
    SIDX = din("SIDX", [128, Ec // 16], i16)
    IND = din("IND", [128, Ec], f8)
    INDT = din("INDT", [128, Ec], f8)
    YATT = din("YATT", [128, T * 4], f32)
    EFLT = din("EFLT", [16, Ec], f16)
    WFS = din("WFS", [MUL, MUL], f16)
    WFR = din("WFR", [MUL, MUL], f16)
    W1CD = din("W1CD", [16, MUL], f16)
    W2 = din("W2", [MUL, MUL], f16)
    W3F = din("W3F", [MUL, 4 * MUL], f16)
    WUP0 = din("WUP0", [MUL, MUL], f16)
    WUP1 = din("WUP1", [MUL, MUL], f16)
    WOSA = din("WOSA", [MUL, MUL], f16)
    WOSB = din("WOSB", [MUL, MUL], f16)
    WOVA = din("WOVA", [MUL, MUL], f16)
    WOVB = din("WOVB", [MUL, MUL], f16)

    OUT = nc.dram_tensor("out", [G * 128, 4 * MUL], f32, kind="ExternalOutput").ap()

    with tile.TileContext(nc) as tc:
        with tc.tile_pool(name="const", bufs=1) as cpool:
            iota_row = cpool.tile([128, 128], f16, tag="iota")
            nc.gpsimd.iota(iota_row[:], pattern=[[1, 128]], base=0,
                           channel_multiplier=0,
                           allow_small_or_imprecise_dtypes=True)
            iota_col = cpool.tile([128, 1], f32, tag="iotac")
            nc.gpsimd.iota(iota_col[:], pattern=[[0, 1]], base=0,
                           channel_multiplier=1,
                           allow_small_or_imprecise_dtypes=True)
            ident = cpool.tile([128, 128], f16, tag="ident")
            nc.vector.tensor_scalar(ident[:], iota_row[:], iota_col[:], None,
                                    OP.is_equal)

            def load(name, ap, p, w, dt):
                t = cpool.tile([p, w], dt, tag=name)
                nc.sync.dma_start(t[:], ap)
                return t

            wfs = load("wfs", WFS, 128, 128, f16)
            wfr = load("wfr", WFR, 128, 128, f16)
            w1cd = load("w1cd", W1CD, 16, 128, f16)
            w2 = load("w2", W2, 128, 128, f16)
            w3f = load("w3f", W3F, 128, 512, f16)
            wup0 = load("wup0", WUP0, 128, 128, f16)
            wup1 = load("wup1", WUP1, 128, 128, f16)
            wosa = load("wosa", WOSA, 128, 128, f16)
            wosb = load("wosb", WOSB, 128, 128, f16)
            wova = load("wova", WOVA, 128, 128, f16)
            wovb = load("wovb", WOVB, 128, 128, f16)
            sidx = load("sidx", SIDX, 128, Ec // 16, i16)
            yatt = load("yatt", YATT, 128, T * 4, f32)


            # ---------------- edge phase pools (table coexists) ----------------
            with tc.tile_pool(name="tbl", bufs=1) as tbl, \
                 tc.tile_pool(name="tblo", bufs=2) as tblo, \
                 tc.tile_pool(name="gatherA", bufs=3) as gpool, \
                 tc.tile_pool(name="gatherB", bufs=2) as gpoolb, \
                 tc.tile_pool(name="edge", bufs=2) as epool, \
                 tc.tile_pool(name="mji", bufs=2) as mjpool, \
                 tc.tile_pool(name="gout", bufs=2) as gopool:

                CH = 768
                ngrp = 128

                # ------- table phase: VUP = V16 @ WUP1, staged write-out -------
                v16t = []
                for i in range(3):
                    vt = tbl.tile([128, N], f16, tag=f"v16t{i}")
                    nc.sync.dma_start(vt[:], V16T[128 * i:128 * (i + 1), :])
                    v16t.append(vt)
                vup_v = CONC.rearrange("(c p) f -> p c f", p=128)[:, :, MUL:]
                SCH = 8                      # staged chunks per write-out
                tbl_ps_cm = tc.tile_pool(name="tbl_ps", bufs=4, space="PSUM")
                tbl_ps = tbl_ps_cm.__enter__()
                for ch0 in range(0, NCH, SCH):
                    chn = min(SCH, NCH - ch0)
                    vupo = tblo.tile([128, SCH, 3 * MUL], f16, tag="vupo")
                    for ck in range(chn):
                        ch = ch0 + ck
                        c0 = ch * 128
                        cn = min(128, N - c0)
                        for i in range(3):
                            ps = tbl_ps.tile([128, 128], f32, tag="tps")
                            nc.tensor.matmul(ps[:cn, :], v16t[i][:, c0:c0 + cn],
                                             wup1[:], start=True, stop=True)
                            dst = vupo[:cn, ck, 128 * i:128 * (i + 1)]
                            if (ch * 3 + i) % 2 == 0:
                                nc.vector.tensor_copy(dst, ps[:cn, :])
                            else:
                                nc.scalar.activation(dst, ps[:cn, :], AF.Copy)
                    full = chn if ch0 + chn < NCH else chn - 1
                    if full:
                        nc.sync.dma_start(vup_v[:, ch0:ch0 + full, :],
                                          vupo[:, :full, :])
                    if ch0 + chn == NCH:
                        nc.sync.dma_start(vup_v[:NLAST, NCH - 1, :],
                                          vupo[:NLAST, chn - 1, :])
                tbl_ps_cm.__exit__(None, None, None)

                with tc.tile_pool(name="hps", bufs=2, space="PSUM") as hps, \
                     tc.tile_pool(name="tpps", bufs=1, space="PSUM") as tpps, \
                     tc.tile_pool(name="sups", bufs=1, space="PSUM") as sups, \
                     tc.tile_pool(name="msgps", bufs=1, space="PSUM") as msgps, \
                     tc.tile_pool(name="trps", bufs=1, space="PSUM") as trps:
                 for g in range(G):
                    e0 = g * EPG
                    i0 = e0 // 16
                    cg = gpool.tile([128, TPG, 512], f16, tag="cg")
                    sstg = gpoolb.tile([128, TPG, 128], f16, tag="sstg")
                    for ci, c0 in enumerate(range(0, EPG, CH)):
                        cn = min(CH, EPG - c0)
                        ic0, icn = i0 + c0 // 16, cn // 16
                        t0, tn = c0 // 128, cn // 128
                        nc.gpsimd.dma_gather(cg[:, t0:t0 + tn, :], CONC,
                                             sidx[:, ic0:ic0 + icn], cn, cn,
                                             4 * MUL, transpose=False, queue_num=0,
                                             single_packet=False)
                        CHT = CH // 128
                        strp = trps.tile([128, CHT, 128], f16,
                                         tag="strA" if ci % 2 == 0 else "strB")
                        for tt in range(tn):
                            nc.tensor.transpose(strp[:, tt, :],
                                                cg[:, t0 + tt, 0:128], ident[:])
                        if ci % 2 == 0:
                            nc.scalar.activation(sstg[:, t0:t0 + tn, :],
                                                 strp[:, :tn, :], AF.Copy)
                        else:
                            nc.vector.tensor_copy(sstg[:, t0:t0 + tn, :],
                                                  strp[:, :tn, :])
                    eflg = gpoolb.tile([16, EPG], f16, tag="eflg")
                    nc.sync.dma_start(eflg[:], EFLT[:, e0:e0 + EPG])
                    indg = gpoolb.tile([128, EPG], f8, tag="indg")
                    nc.sync.dma_start(indg[:], IND[:, e0:e0 + EPG])
                    indTg = gpoolb.tile([128, EPG], f8, tag="indTg")
                    nc.sync.dma_start(indTg[:], INDT[:, e0:e0 + EPG])

                    # per-group receiver-scalar table: ZGT[n, m]
                    s16tg = gpoolb.tile([128, 128], f16, tag="s16tg")
                    nc.sync.dma_start(s16tg[:, :ngrp],
                                      S16TG[:, g * 128:g * 128 + ngrp])
                    zg_ps = hps.tile([128, 128], f32, tag="h")
                    nc.tensor.matmul(zg_ps[:ngrp, :], s16tg[:, :ngrp], wfr[:],
                                     start=True, stop=True)
                    zgt = gpoolb.tile([128, 128], f16, tag="zgt")
                    nc.vector.tensor_copy(zgt[:ngrp, :], zg_ps[:ngrp, :])

                    msg = msgps.tile([128, 1024], f32, tag="msg")
                    yat4 = yatt[:].rearrange("p (t k) -> p t k", k=4)

                    BT = 4                         # tiles per vector block
                    for b0 in range(0, TPG, BT):
                        bn = min(BT, TPG - b0)
                        twq = epool.tile([128, BT, 512], f16, tag="twq")
                        supq_ps = sups.tile([128, BT, 128], f32, tag="supq")
                        for jj in range(bn):
                            j = b0 + jj
                            es = j * 128
                            h1 = hps.tile([128, 128], f32, tag="h")
                            nc.tensor.matmul(h1[:], wfs[:], sstg[:, j, :],
                                             start=True, stop=False)
                            nc.tensor.matmul(h1[:], w1cd[:], eflg[:, es:es + 128],
                                             start=False, stop=False)
                            nc.tensor.matmul(h1[:], zgt[:ngrp, :],
                                             indTg[:ngrp, es:es + 128],
                                             start=False, stop=True)
                            h1s = epool.tile([128, 128], f16, tag="h1s")
                            nc.scalar.activation(h1s[:], h1[:], AF.Silu)
                            h2 = hps.tile([128, 128], f32, tag="h")
                            nc.tensor.matmul(h2[:], w2[:], h1s[:],
                                             start=True, stop=True)
                            h2s = epool.tile([128, 128], f16, tag="h2s")
                            nc.scalar.activation(h2s[:], h2[:], AF.Silu)
                            tp = tpps.tile([128, 512], f32, tag="tp")
                            nc.tensor.matmul(tp[:], h2s[:], w3f[:],
                                             start=True, stop=True)
                            nc.scalar.activation(twq[:, jj, :], tp[:], AF.Copy)
                            nc.tensor.matmul(supq_ps[:, jj, :],
                                             sstg[:, j, :], wup0[:],
                                             start=True, stop=True)
                        supq = epool.tile([128, BT, 128], f16, tag="supq")
                        nc.scalar.activation(supq[:, :bn, :], supq_ps[:, :bn, :],
                                             AF.Copy)

                        t0 = g * TPG + b0
                        y0q = yat4[:, t0:t0 + bn, 0:1]          # [p, bn, 1]
                        y1q = yat4[:, t0:t0 + bn, 1:4]          # [p, bn, 3]
                        vgq = cg[:, b0:b0 + bn, 128:512]        # [p, bn, 384]

                        # vd = sum_i v_i * y1_i   (vtmp layout [p, bn, u, i])
                        vtmp = epool.tile([128, BT, 128, 3], f16, tag="vtmp")
                        nc.vector.tensor_tensor(
                            vtmp[:, :bn], vgq.rearrange("p t (i u) -> p t u i", i=3),
                            y1q.unsqueeze(2).broadcast_to([128, bn, 128, 3]),
                            OP.mult)
                        vd = epool.tile([128, BT, 128], f32, tag="vd")
                        nc.vector.tensor_reduce(vd[:, :bn], vtmp[:, :bn],
                                                mybir.AxisListType.X, OP.add)
                        # supy = sup*y0 ; y0w2 = w2*y0 ; sw1 = sup*w1
                        supy = epool.tile([128, BT, 128], f16, tag="supy")
                        nc.vector.tensor_tensor(
                            supy[:, :bn], supq[:, :bn, :],
                            y0q.broadcast_to([128, bn, 128]), OP.mult)
                        y0w2 = epool.tile([128, BT, 128], f16, tag="y0w2")
                        nc.vector.tensor_tensor(
                            y0w2[:, :bn], twq[:, :bn, 256:384],
                            y0q.broadcast_to([128, bn, 128]), OP.mult)
                        sw1 = epool.tile([128, BT, 128], f16, tag="sw1")
                        nc.vector.tensor_tensor(sw1[:, :bn], supq[:, :bn, :],
                                                twq[:, :bn, 128:256], OP.mult)

                        mj = mjpool.tile([128, BT, 1024], f16, tag="mj")
                        mj4 = mj[:].rearrange("p t (i u) -> p t i u", u=128)
                        nc.vector.tensor_tensor(mj4[:, :bn, 0, :], supy[:, :bn],
                                                twq[:, :bn, 0:128], OP.mult)
                        nc.vector.tensor_tensor(mj4[:, :bn, 1, :], vd[:, :bn],
                                                twq[:, :bn, 384:512], OP.mult)
                        nc.vector.tensor_tensor(
                            mj4[:, :bn, 2:5, :],
                            sw1[:, :bn].unsqueeze(2).broadcast_to([128, bn, 3, 128]),
                            y1q.unsqueeze(3).broadcast_to([128, bn, 3, 128]),
                            OP.mult)
                        nc.vector.tensor_tensor(
                            mj4[:, :bn, 5:8, :],
                            vgq.rearrange("p t (i u) -> p t i u", i=3),
                            y0w2[:, :bn].unsqueeze(2).broadcast_to([128, bn, 3, 128]),
                            OP.mult)

                        for jj in range(bn):
                            j = b0 + jj
                            es = j * 128
                            nc.tensor.matmul(msg[:, 0:512], indg[:, es:es + 128],
                                             mj[:, jj, 0:512],
                                             start=(j == 0), stop=(j == TPG - 1))
                            nc.tensor.matmul(msg[:, 512:1024], indg[:, es:es + 128],
                                             mj[:, jj, 512:1024],
                                             start=(j == 0), stop=(j == TPG - 1))

                    # ---------------- group epilogue ----------------
                    msgs = gopool.tile([128, 1024], f16, tag="msgs")
                    nc.vector.tensor_copy(msgs[:], msg[:])
                    mts = []
                    for b in range(8):
                        mt_ps = trps.tile([128, CH // 128, 128], f16,
                                          tag="strA" if b % 2 == 0 else "strB")
                        nc.tensor.transpose(mt_ps[:, 0, :],
                                            msgs[:, 128 * b:128 * (b + 1)],
                                            ident[:])
                        mt = gopool.tile([128, 128], f16, tag=f"mt{b}")
                        if b % 2 == 0:
                            nc.vector.tensor_copy(mt[:], mt_ps[:, 0, :])
                        else:
                            nc.scalar.activation(mt[:], mt_ps[:, 0, :], AF.Copy)
                        mts.append(mt)
                    wo = tpps.tile([128, 512], f32, tag="tp")
                    nc.tensor.matmul(wo[:, 0:128], mts[0][:], wosa[:],
                                     start=True, stop=False)
                    nc.tensor.matmul(wo[:, 0:128], mts[1][:], wosb[:],
                                     start=False, stop=True)
                    for i in range(3):
                        nc.tensor.matmul(wo[:, 128 * (1 + i):128 * (2 + i)],
                                         mts[2 + i][:], wova[:],
                                         start=True, stop=False)
                        nc.tensor.matmul(wo[:, 128 * (1 + i):128 * (2 + i)],
                                         mts[5 + i][:], wovb[:],
                                         start=False, stop=True)
                    asm = gopool.tile([128, 512], f32, tag="asm")
                    asm4 = asm[:].rearrange("p (u k) -> p u k", k=4)
                    wo4 = wo[:].rearrange("p (k u) -> p k u", k=4)
                    for k in range(4):
                        if k % 2 == 0:
                            nc.vector.tensor_scalar(asm4[:, :, k], wo4[:, k, :],
                                                    1.0 / W3_SCALE, None, OP.mult)
                        else:
                            nc.scalar.activation(asm4[:, :, k], wo4[:, k, :],
                                                 AF.Copy, scale=1.0 / W3_SCALE)
                    nc.sync.dma_start(OUT[g * 128:g * 128 + ngrp, :],
                                      asm[:ngrp, :])

    nc.compile()
    return nc


def kernel(**inputs):
    meta, in_maps = _prep(**inputs)
    key = (meta["TPG"], meta["G"])
    if key not in _COMPILED:
        _COMPILED[key] = _build(meta)
    nc = _COMPILED[key]

    from concourse.bass_utils import run_bass_kernel_spmd
    res = run_bass_kernel_spmd(nc, in_maps, list(range(N_CORES)))
    outs = [res.results[c]["out"] for c in range(N_CORES)]
    full = np.concatenate(outs, axis=0)[meta["out_row"]]
    return full.reshape(N_NODES, MUL, 4).astype(np.float32)
